# revision 47
# baseline (speedup 1.0000x reference)
"""Trainium2 Bass kernel for nn_Attention_Net (nms_detection).

Data-parallel over batch: 8 batches -> 8 NeuronCores. Per core:
  1. proposal head: conv3x3(2048->128) + 2x conv3x3(128->128, last stride2)
     + three 1x1 tidy convs -> 2793 anchor scores.
  2. greedy NMS top-4 (argmax / IoU>=0.5 suppress, 4 rounds) on device.
  3. bilinear crop_resize (224x224, align-corners style with integer boxes)
     of the zero-padded image for the 4 picked boxes, via indirect-DMA row
     gather + ap_gather column gather.

Host side only reshapes/transposes inputs (layout prep) and concatenates
outputs; all math runs on the NeuronCores.
"""

import math
import os
import sys

import numpy as np

sys.path.insert(0, "/opt/trn_rl_repo")

import concourse.bass as bass
import concourse.tile as tile
from concourse import mybir
from concourse.bass import AP, IndirectOffsetOnAxis
from concourse.bass_utils import run_bass_kernel_spmd

F32 = mybir.dt.float32
I32 = mybir.dt.int32
I16 = mybir.dt.int16

IMG = 448
PAD = 224
OUT = 224
TOPN = 4
NCORES = 8
A = 2793          # anchors
SR, SF = 21, 196  # on-device score layout [21, 196]
NEG = -float("inf")
BIG = 1.0e9
IOTA_PAD = 99999.0

Alu = mybir.AluOpType
Act = mybir.ActivationFunctionType


# ---------------------------------------------------------------- host prep
def _flat_to_rf():
    """flat anchor index -> (row, col) in the [21,196] device layout."""
    rows = np.zeros(A, np.int64)
    cols = np.zeros(A, np.int64)
    i = np.arange(A)
    m1 = i < 1176
    rows[m1] = i[m1] // 196
    cols[m1] = i[m1] % 196
    m2 = (i >= 1176) & (i < 2352)
    rows[m2] = 6 + (i[m2] - 1176) // 196
    cols[m2] = (i[m2] - 1176) % 196
    m3 = i >= 2352
    rows[m3] = 12 + (i[m3] - 2352) // 49
    cols[m3] = (i[m3] - 2352) % 49
    return rows, cols


def _consts(edge_anchors):
    rows, cols = _flat_to_rf()
    boxes = edge_anchors.astype(np.float32)  # [A,4] y0 x0 y1 x1
    planes = np.zeros((4, SR, SF), np.float32)
    for c in range(4):
        planes[c][rows, cols] = boxes[:, c]
    a2 = (boxes[:, 2] - boxes[:, 0]) * (boxes[:, 3] - boxes[:, 1])
    a2p = np.zeros((SR, SF), np.float32)
    a2p[rows, cols] = a2
    iota_gl = np.full((SR, SF), IOTA_PAD, np.float32)
    iota_gl[rows, cols] = np.arange(A, dtype=np.float32)

    id128 = np.eye(128, dtype=np.float32)
    ones1x128 = np.ones((1, 128), np.float32)
    ones8 = np.ones((8, 112), np.float32)
    p = np.arange(128)[:, None]
    m = np.arange(112)[None, :]
    # blk464[c, p] = 1 if p//16 == c (broadcast crop scalar to its 16-group)
    blk464 = (np.arange(64)[None, :] // 16 == np.arange(4)[:, None]).astype(
        np.float32
    )
    trow = np.broadcast_to(
        (np.arange(OUT, dtype=np.float32) / np.float32(OUT - 1))[None, :], (4, OUT)
    ).copy()
    # x-index chain, wrapped [16,28] layout replicated to 4 crops [64,28]
    s = np.arange(28)[None, :]
    pp = np.arange(64)[:, None]
    kk = 16 * (s % 14) + (pp % 16)
    txw = np.broadcast_to((kk.astype(np.float32) / np.float32(OUT - 1)), (64, 28)).copy()
    addw = np.broadcast_to((s >= 14).astype(np.float32), (64, 28)).copy()
    # weight chain [4,448]
    k448 = np.arange(448)[None, :]
    txd = np.broadcast_to(
        ((k448 % 224).astype(np.float32) / np.float32(OUT - 1)), (4, 448)
    ).copy()
    addd = np.broadcast_to((k448 >= 224).astype(np.float32), (4, 448)).copy()
    aw = np.broadcast_to((k448 < 224).astype(np.float32), (4, 448)).copy()
    # pick4[j, k*112+m] = (j == k): broadcast row k of a [4,*] tile to 112 parts
    jj = np.arange(4)[:, None]
    km = np.arange(448)[None, :]
    pick4 = (jj == (km // 112)).astype(np.float32)
    pq = np.arange(112)[:, None]
    qq = np.arange(4)[None, :]
    uofs = (-(224.0 + 112.0 * qq + pq)).astype(np.float32)
    # eye128_4[p, 4k+j] = (p==0)&(j==k): select broadcast row into selr row k
    eye128_4 = np.zeros((128, 16), np.float32)
    for k_ in range(4):
        eye128_4[0, 4 * k_ + k_] = 1.0
    # pickrep[p, k*112+m] = (p//16 == k) & (p%16 == m%16): replicate crop-k
    # 16-row wrapped tile to 112 partitions
    pp64 = np.arange(64)[:, None]
    pickrep = (((pp64 // 16) == (km // 112)) & ((pp64 % 16) == (km % 112 % 16))).astype(np.float32)
    bw = np.broadcast_to(
        np.where(k448 < 224, -1.0, 1.0).astype(np.float32), (4, 448)
    ).copy()
    return dict(
        y0pl=planes[0],
        x0pl=planes[1],
        y1pl=planes[2],
        x1pl=planes[3],
        a2pl=a2p,
        iota_gl=iota_gl,
        id128=id128,
        ones1x128=ones1x128,
        ones8=ones8,
        blk464=blk464,
        trow=trow,
        pick4=pick4,
        eye128_4=eye128_4,
        uofs=uofs,
    )


def _prep_weights(w1, b1, w2, b2, w3, b3, tw1, tb1, tw2, tb2, tw3, tb3):
    rows, cols = _flat_to_rf()
    # w1 [128,2048,3,3] -> [16, 128i, 9, 128o]
    w1p = np.ascontiguousarray(
        w1.reshape(128, 16, 128, 9).transpose(1, 2, 3, 0)
    ).astype(np.float32)
    w2p = np.ascontiguousarray(w2.reshape(128, 128, 9).transpose(1, 2, 0)).astype(
        np.float32
    )
    w3p = np.ascontiguousarray(w3.reshape(128, 128, 9).transpose(1, 2, 0)).astype(
        np.float32
    )
    tw1p = np.zeros((128, SR), np.float32)
    tw1p[:, 0:6] = tw1.reshape(6, 128).T
    tw2p = np.zeros((128, SR), np.float32)
    tw2p[:, 6:12] = tw2.reshape(6, 128).T
    tw3p = np.zeros((128, SR), np.float32)
    tw3p[:, 12:21] = tw3.reshape(9, 128).T
    # bias + (-inf at pad positions) plane in the [21,196] layout
    tbcat = np.concatenate([
        np.repeat(tb1.astype(np.float32), 1), np.repeat(tb2.astype(np.float32), 1),
        np.repeat(tb3.astype(np.float32), 1)])  # [21]
    biaspad = np.full((SR, SF), NEG, np.float32)
    biaspad[rows, cols] = tbcat[rows]
    return dict(
        w1p=w1p,
        w2p=w2p,
        w3p=w3p,
        tw1p=tw1p,
        tw2p=tw2p,
        tw3p=tw3p,
        b1c=b1.reshape(128, 1).astype(np.float32),
        b2c=b2.reshape(128, 1).astype(np.float32),
        b3c=b3.reshape(128, 1).astype(np.float32),
        biaspad=biaspad,
    )


# ---------------------------------------------------------------- device prog
def build_nc():
    nc = bass.Bass()

    # inputs
    rpn = nc.dram_tensor("rpn", [16, 128, 256], F32, kind="ExternalInput")
    xhcw = nc.dram_tensor("xhcw", [448, 1344], F32, kind="ExternalInput")
    w1t = nc.dram_tensor("w1p", [16, 128, 9, 128], F32, kind="ExternalInput")
    w2t = nc.dram_tensor("w2p", [128, 9, 128], F32, kind="ExternalInput")
    w3t = nc.dram_tensor("w3p", [128, 9, 128], F32, kind="ExternalInput")
    tw1t = nc.dram_tensor("tw1p", [128, SR], F32, kind="ExternalInput")
    tw2t = nc.dram_tensor("tw2p", [128, SR], F32, kind="ExternalInput")
    tw3t = nc.dram_tensor("tw3p", [128, SR], F32, kind="ExternalInput")
    bpt = nc.dram_tensor("biaspad", [SR, SF], F32, kind="ExternalInput")
    b1t = nc.dram_tensor("b1c", [128, 1], F32, kind="ExternalInput")
    b2t = nc.dram_tensor("b2c", [128, 1], F32, kind="ExternalInput")
    b3t = nc.dram_tensor("b3c", [128, 1], F32, kind="ExternalInput")
    cns = {}
    for name, shp in [
        ("y0pl", [SR, SF]),
        ("x0pl", [SR, SF]),
        ("y1pl", [SR, SF]),
        ("x1pl", [SR, SF]),
        ("a2pl", [SR, SF]),
        ("iota_gl", [SR, SF]),
        ("id128", [128, 128]),
        ("ones1x128", [1, 128]),
        ("ones8", [8, 112]),
        ("blk464", [4, 64]),
        ("trow", [4, OUT]),
        ("pick4", [4, 448]),
        ("eye128_4", [128, 16]),
        ("uofs", [112, 4]),
    ]:
        cns[name] = nc.dram_tensor(name, shp, F32, kind="ExternalInput")

    # outputs
    part_out = nc.dram_tensor("part", [TOPN, 3, OUT, OUT], F32, kind="ExternalOutput")
    prob_out = nc.dram_tensor("prob", [1, TOPN], F32, kind="ExternalOutput")
    idx_out = nc.dram_tensor("idx", [1, TOPN], I32, kind="ExternalOutput")

    with tile.TileContext(nc) as tc:
        with (
            tc.tile_pool(name="consts", bufs=1) as cp,
            tc.tile_pool(name="wstream", bufs=6) as wp,
            tc.tile_pool(name="rstream", bufs=16) as rp,
            tc.tile_pool(name="acts", bufs=1) as ap_,
            tc.tile_pool(name="nms", bufs=1) as np_,
            tc.tile_pool(name="nms_it", bufs=1) as ni,
            tc.tile_pool(name="bcast", bufs=4) as bcp,
            tc.tile_pool(name="crop", bufs=1) as crp,
            tc.tile_pool(name="rows", bufs=2) as rwp,
            tc.tile_pool(name="hout", bufs=2) as hop,
            tc.tile_pool(name="ps", bufs=2, space="PSUM") as ps,
            tc.tile_pool(name="ps_big", bufs=2, space="PSUM") as psb,
            tc.tile_pool(name="ps_conv", bufs=1, space="PSUM") as psc,
        ):
            # ---- load constants into SBUF
            c_sb = {}
            for name, t in cns.items():
                shp = list(t.shape)
                tl = cp.tile(shp, F32, tag=name)
                nc.sync.dma_start(tl[:], t[:])
                c_sb[name] = tl
            bias_sb = {}
            for name, t, pdim in [
                ("b1", b1t, 128),
                ("b2", b2t, 128),
                ("b3", b3t, 128),
            ]:
                tl = cp.tile([pdim, 1], F32, tag="bias" + name)
                nc.sync.dma_start(tl[:], t[:])
                bias_sb[name] = tl
            tw_sb = {}
            for name, t in [("tw1", tw1t), ("tw2", tw2t), ("tw3", tw3t)]:
                tl = cp.tile([128, SR], F32, tag="w" + name)
                nc.sync.dma_start(tl[:], t[:])
                tw_sb[name] = tl
            biaspad_sb = cp.tile([SR, SF], F32, tag="biaspad")
            nc.sync.dma_start(biaspad_sb[:], bpt[:])
            imgY = []
            for q in range(4):
                ti = cp.tile([112, 1344], F32, tag=f"imgY{q}")
                nc.sync.dma_start(ti[:], xhcw[112 * q : 112 * q + 112, :])
                imgY.append(ti)
            w2sb = cp.tile([128, 9, 128], F32, tag="w2sb")
            nc.sync.dma_start(w2sb[:], w2t[:])
            w3sb = cp.tile([128, 9, 128], F32, tag="w3sb")
            nc.sync.dma_start(w3sb[:], w3t[:])

            # PE "touch": absorb a DMA wait into the PE vector clock so the
            # subsequent matmul's LDWEIGHTS carries at most one sem wait.
            id1 = c_sb["id128"][0:1, 0:1]
            touch_ps = psc.tile([1, 1], F32, tag="touchps")

            def touch(ap2d):
                nc.tensor.transpose(touch_ps[:], ap2d, id1)

            # same-engine self-copy touches (consumer must be on same engine)
            def act_touch(ap2d):
                nc.scalar.copy(ap2d, ap2d)

            def dve_touch(ap2d):
                nc.vector.tensor_copy(ap2d, ap2d)

            def gp_touch(ap2d):
                nc.gpsimd.tensor_copy(ap2d, ap2d)

            agar = cp.tile([1, 64], F32, tag="agar")
            vgar = cp.tile([1, 64], F32, tag="vgar")
            _tcn = {"a": 0, "v": 0}

            def act_obs(ap2d):
                i = _tcn["a"] % 64
                _tcn["a"] += 1
                nc.scalar.copy(agar[0:1, i : i + 1], ap2d)

            def dve_obs(ap2d):
                i = _tcn["v"] % 64
                _tcn["v"] += 1
                nc.vector.tensor_copy(vgar[0:1, i : i + 1], ap2d)

            for _t in [c_sb["id128"], c_sb["ones1x128"], c_sb["ones8"],
                       c_sb["pick4"], c_sb["blk464"],
                       c_sb["eye128_4"], tw_sb["tw1"], tw_sb["tw2"],
                       tw_sb["tw3"], c_sb["iota_gl"], c_sb["a2pl"],
                       c_sb["y0pl"], c_sb["x0pl"], c_sb["y1pl"], c_sb["x1pl"]]:
                touch(_t[0:1, 0:1])
            touch(w2sb[0:1, 0:1, 0:1])
            touch(w3sb[0:1, 0:1, 0:1])
            for _t in [c_sb["iota_gl"], c_sb["a2pl"], c_sb["y0pl"],
                       c_sb["x0pl"], c_sb["y1pl"], c_sb["x1pl"],
                       biaspad_sb, c_sb["trow"]]:
                dve_obs(_t[0:1, 0:1])
            for _t in [bias_sb["b1"], bias_sb["b2"], bias_sb["b3"]]:
                act_obs(_t[0:1, 0:1])

            # ---- conv1: 16 in-chunks x 9 taps accumulated in PSUM
            w1full = cp.tile([128, 16, 9, 128], F32, tag="w1full")
            for q in range(4):
                nc.sync.dma_start(
                    w1full[:, 4 * q : 4 * q + 4, :, :],
                    w1t[4 * q : 4 * q + 4].rearrange("c i t o -> i c t o"),
                )
            p1 = psc.tile([128, 196], F32, tag="convps")
            first = True
            for c in range(16):
                rpad = rp.tile([128, 16, 16], F32, tag="rpad")
                nc.sync.dma_start(rpad[:], rpn[c])
                touch(rpad[0:1, 0:1, 0:1])
                if c % 4 == 0:
                    touch(w1full[0:1, c : c + 1, 0:1, 0:1])
                for t in range(9):
                    dy, dx = t // 3, t % 3
                    nc.tensor.matmul(
                        p1[:],
                        lhsT=w1full[:, c, t, :],
                        rhs=rpad[:, dy : dy + 14, dx : dx + 14],
                        start=first,
                        stop=(c == 15 and t == 8),
                    )
                    first = False

            d1pad = ap_.tile([128, 16, 16], F32)
            nc.vector.memset(d1pad[:], 0.0)
            touch(d1pad[0:1, 0:1, 0:1])
            act_obs(d1pad[0:1, 0:1, 0:1])
            nc.scalar.activation(
                d1pad[:, 1:15, 1:15], p1[:], Act.Relu, bias=bias_sb["b1"][:]
            )
            touch(d1pad[0:1, 1:2, 1:2])

            # ---- conv2
            p2 = psc.tile([128, 196], F32, tag="convps")
            for t in range(9):
                dy, dx = t // 3, t % 3
                nc.tensor.matmul(
                    p2[:],
                    lhsT=w2sb[:, t, :],
                    rhs=d1pad[:, dy : dy + 14, dx : dx + 14],
                    start=(t == 0),
                    stop=(t == 8),
                )
            d2pad = ap_.tile([128, 16, 16], F32)
            nc.vector.memset(d2pad[:], 0.0)
            touch(d2pad[0:1, 0:1, 0:1])
            act_obs(d2pad[0:1, 0:1, 0:1])
            nc.scalar.activation(
                d2pad[:, 1:15, 1:15], p2[:], Act.Relu, bias=bias_sb["b2"][:]
            )
            touch(d2pad[0:1, 1:2, 1:2])

            # ---- conv3 stride 2 -> [128, 49]
            p3 = psc.tile([128, 49], F32, tag="convps")
            d2v = d2pad[:].rearrange("p (a c) (b d) -> p a c b d", a=8, c=2, b=8, d=2)
            for t in range(9):
                dy, dx = t // 3, t % 3
                rhs = d2v[
                    :,
                    dy // 2 : dy // 2 + 7,
                    dy % 2 : dy % 2 + 1,
                    dx // 2 : dx // 2 + 7,
                    dx % 2 : dx % 2 + 1,
                ]
                nc.tensor.matmul(
                    p3[:], lhsT=w3sb[:, t, :], rhs=rhs, start=(t == 0), stop=(t == 8)
                )
            d3 = ap_.tile([128, 49], F32)
            nc.scalar.activation(d3[:], p3[:], Act.Relu, bias=bias_sb["b3"][:])

            # ---- tidy 1x1 convs -> scores [21,196] (padded-col weights,
            #      accumulated into a single PSUM tile, then +bias/-inf plane)
            pt_all = ps.tile([SR, SF], F32, tag="pt")
            nc.tensor.matmul(
                pt_all[:], lhsT=tw_sb["tw1"][:], rhs=d1pad[:, 1:15, 1:15],
                start=True, stop=False,
            )
            nc.tensor.matmul(
                pt_all[:], lhsT=tw_sb["tw2"][:], rhs=d2pad[:, 1:15, 1:15],
                start=False, stop=False,
            )
            touch(d3[0:1, 0:1])
            nc.tensor.matmul(
                pt_all[:, 0:49], lhsT=tw_sb["tw3"][:], rhs=d3[:],
                start=False, stop=True,
            )
            scores = np_.tile([SR, SF], F32, tag="scores")
            nc.vector.tensor_add(scores[:], pt_all[:], biaspad_sb[:])
            sc_orig = np_.tile([SR, SF], F32, tag="sc_orig")
            nc.vector.tensor_scalar(
                sc_orig[:], scores[:], -1.0e30, None, op0=Alu.max
            )
            neginf = np_.tile([SR, SF], F32, tag="neginf")
            nc.vector.memset(neginf[:], NEG)

            iota_gl = c_sb["iota_gl"]
            id128 = c_sb["id128"]
            ones8 = c_sb["ones8"]

            probrow = np_.tile([1, TOPN], F32, tag="probrow")
            idxrow = np_.tile([1, TOPN], F32, tag="idxrow")

            scr = np_.tile([SR, SF], F32, tag="scr")  # scratch for TTR
            sel21 = np_.tile([SR, 5], F32, tag="sel21")

            bc128s = []  # per-pick broadcast scalars [128,5]: y0 x0 y1 x1 prob
            selr_ps = ps.tile([4, 5], F32, tag="pt")

            planes = [c_sb["y0pl"], c_sb["x0pl"], c_sb["y1pl"], c_sb["x1pl"], sc_orig]

            for k in range(TOPN):
                # -------- global argmax (first occurrence, flat order)
                m21 = ni.tile([SR, 1], F32, tag="m21")
                nc.vector.reduce_max(m21[:], scores[:], axis=mybir.AxisListType.X)
                touch(m21[0:1, 0:1])
                mt = ps.tile([1, SR], F32, tag="small")
                nc.tensor.transpose(mt[:], m21[:], id128[0:SR, 0:SR])
                mrow = ni.tile([1, SR], F32, tag="mrow")
                nc.vector.tensor_copy(mrow[:], mt[:])
                m11 = ni.tile([1, 1], F32, tag="m11")
                nc.vector.reduce_max(m11[:], mrow[:], axis=mybir.AxisListType.X)
                touch(m11[0:1, 0:1])
                mb_ps = ps.tile([SR, 1], F32, tag="small")
                nc.tensor.matmul(
                    mb_ps[:], lhsT=ones8[0:1, 0:SR], rhs=m11[:], start=True, stop=True
                )
                mb = ni.tile([SR, 1], F32, tag="mb")
                nc.vector.tensor_copy(mb[:], mb_ps[:])
                eq = ni.tile([SR, SF], F32, tag="eq")
                nc.vector.tensor_scalar(
                    eq[:], scores[:], mb[:], None, op0=Alu.is_equal
                )
                pen = ni.tile([SR, SF], F32, tag="pen")
                nc.vector.tensor_scalar(
                    pen[:], eq[:], -BIG, BIG, op0=Alu.mult, op1=Alu.add
                )
                cand = ni.tile([SR, SF], F32, tag="cand")
                nc.vector.tensor_add(cand[:], iota_gl[:], pen[:])
                c21 = ni.tile([SR, 1], F32, tag="c21")
                nc.vector.tensor_reduce(
                    c21[:], cand[:], axis=mybir.AxisListType.X, op=Alu.min
                )
                touch(c21[0:1, 0:1])
                ct = ps.tile([1, SR], F32, tag="small")
                nc.tensor.transpose(ct[:], c21[:], id128[0:SR, 0:SR])
                crow = ni.tile([1, SR], F32, tag="crow")
                nc.vector.tensor_copy(crow[:], ct[:])
                g11 = ni.tile([1, 1], F32, tag="g11")
                nc.vector.tensor_reduce(
                    g11[:], crow[:], axis=mybir.AxisListType.X, op=Alu.min
                )
                nc.vector.tensor_copy(idxrow[0:1, k : k + 1], g11[:])
                touch(g11[0:1, 0:1])
                gb_ps = ps.tile([SR, 1], F32, tag="small")
                nc.tensor.matmul(
                    gb_ps[:], lhsT=ones8[0:1, 0:SR], rhs=g11[:], start=True, stop=True
                )
                gb = ni.tile([SR, 1], F32, tag="gb")
                nc.vector.tensor_copy(gb[:], gb_ps[:])
                onehot = ni.tile([SR, SF], F32, tag="onehot")
                nc.vector.tensor_scalar(
                    onehot[:], iota_gl[:], gb[:], None, op0=Alu.is_equal
                )

                # -------- extract box coords + prob of the pick
                for j, pl in enumerate(planes):
                    nc.vector.tensor_mul(scr[:], onehot[:], pl[:])
                    nc.vector.tensor_reduce(
                        sel21[:, j : j + 1], scr[:],
                        axis=mybir.AxisListType.X, op=Alu.add,
                    )
                touch(sel21[0:1, 4:5])
                st = ps.tile([5, SR], F32, tag="small")
                nc.tensor.transpose(st[:], sel21[:], id128[0:SR, 0:SR])
                s21 = ni.tile([5, SR], F32, tag="s21")
                nc.vector.tensor_copy(s21[:], st[:])
                sel5 = ni.tile([5, 1], F32, tag="sel5")
                nc.vector.tensor_reduce(
                    sel5[:], s21[:], axis=mybir.AxisListType.X, op=Alu.add
                )
                touch(sel5[0:1, 0:1])
                srow_ps = ps.tile([1, 5], F32, tag="small")
                nc.tensor.transpose(srow_ps[:], sel5[:], id128[0:5, 0:5])
                srow = ni.tile([1, 5], F32, tag="srow")
                nc.vector.tensor_copy(srow[:], srow_ps[:])
                nc.vector.tensor_copy(probrow[0:1, k : k + 1], srow[0:1, 4:5])
                touch(srow[0:1, 0:1])
                bc_ps = psb.tile([128, 5], F32, tag="big")
                nc.tensor.matmul(
                    bc_ps[:],
                    lhsT=c_sb["ones1x128"][:],
                    rhs=srow[:],
                    start=True,
                    stop=True,
                )
                bc = bcp.tile([128, 5], F32, tag="bc")
                nc.vector.tensor_copy(bc[:], bc_ps[:])
                bc128s.append(bc)
                touch(bc[0:1, 0:1])
                nc.tensor.matmul(
                    selr_ps[:],
                    lhsT=c_sb["eye128_4"][:, 4 * k : 4 * k + 4],
                    rhs=bc[:],
                    start=(k == 0),
                    stop=(k == TOPN - 1),
                )

                # -------- IoU suppress (skip after last pick)
                if k == TOPN - 1:
                    break
                yy0 = ni.tile([SR, SF], F32, tag="yy0")
                nc.vector.tensor_scalar(
                    yy0[:], planes[0][:], bc[0:SR, 0:1], None, op0=Alu.max
                )
                xx0 = ni.tile([SR, SF], F32, tag="xx0")
                nc.vector.tensor_scalar(
                    xx0[:], planes[1][:], bc[0:SR, 1:2], None, op0=Alu.max
                )
                yy1 = ni.tile([SR, SF], F32, tag="yy1")
                nc.vector.tensor_scalar(
                    yy1[:], planes[2][:], bc[0:SR, 2:3], None, op0=Alu.min
                )
                xx1 = ni.tile([SR, SF], F32, tag="xx1")
                nc.vector.tensor_scalar(
                    xx1[:], planes[3][:], bc[0:SR, 3:4], None, op0=Alu.min
                )
                hh = ni.tile([SR, SF], F32, tag="hh")
                nc.vector.tensor_sub(hh[:], yy1[:], yy0[:])
                nc.vector.tensor_scalar(hh[:], hh[:], 0.0, None, op0=Alu.max)
                ww = ni.tile([SR, SF], F32, tag="ww")
                nc.vector.tensor_sub(ww[:], xx1[:], xx0[:])
                nc.vector.tensor_scalar(ww[:], ww[:], 0.0, None, op0=Alu.max)
                inter = ni.tile([SR, SF], F32, tag="inter")
                nc.vector.tensor_mul(inter[:], hh[:], ww[:])
                # a1 scalar = (y1-y0)*(x1-x0) of pick
                bh = ni.tile([128, 1], F32, tag="bh")
                nc.vector.tensor_sub(bh[:], bc[:, 2:3], bc[:, 0:1])
                bw_ = ni.tile([128, 1], F32, tag="bw_")
                nc.vector.tensor_sub(bw_[:], bc[:, 3:4], bc[:, 1:2])
                a1 = ni.tile([128, 1], F32, tag="a1")
                nc.vector.tensor_mul(a1[:], bh[:], bw_[:])
                den = ni.tile([SR, SF], F32, tag="den")
                nc.vector.tensor_scalar(
                    den[:], c_sb["a2pl"][:], a1[0:SR, 0:1], None, op0=Alu.add
                )
                nc.vector.tensor_sub(den[:], den[:], inter[:])
                # suppress iff (den>=0) & (2*inter-den>=0) & (den + (2*inter-den) > 0)
                s_ = ni.tile([SR, SF], F32, tag="s_")
                nc.vector.tensor_scalar(
                    s_[:], inter[:], 2.0, None, op0=Alu.mult
                )
                nc.vector.tensor_sub(s_[:], s_[:], den[:])
                m1_ = ni.tile([SR, SF], F32, tag="m1_")
                nc.vector.tensor_scalar(m1_[:], den[:], 0.0, None, op0=Alu.is_ge)
                m2_ = ni.tile([SR, SF], F32, tag="m2_")
                nc.vector.tensor_scalar(m2_[:], s_[:], 0.0, None, op0=Alu.is_ge)
                ds_ = ni.tile([SR, SF], F32, tag="ds_")
                nc.vector.tensor_add(ds_[:], den[:], s_[:])
                m3_ = ni.tile([SR, SF], F32, tag="m3_")
                nc.vector.tensor_scalar(m3_[:], ds_[:], 0.0, None, op0=Alu.is_gt)
                nc.vector.tensor_mul(m1_[:], m1_[:], m2_[:])
                mask_i = ni.tile([SR, SF], mybir.dt.uint8, tag="mask_i")
                nc.vector.tensor_mul(mask_i[:], m1_[:], m3_[:])
                nc.vector.copy_predicated(scores[:], mask_i[:], neginf[:])

            # ================= crop phase =================
            # per-crop scalars: sel rows stacked [4,5]
            selr = crp.tile([4, 5], F32, tag="selr")
            nc.vector.tensor_copy(selr[:], selr_ps[:])
            # ---------- y-side: ys rows [4,224] (clipped padded coords)
            sy = crp.tile([4, 1], F32, tag="sy")
            nc.vector.tensor_sub(sy[:], selr[:, 2:3], selr[:, 0:1])
            nc.vector.tensor_scalar(sy[:], sy[:], 1.0, None, op0=Alu.subtract)
            ys = crp.tile([4, OUT], F32, tag="ys")
            nc.vector.tensor_scalar(
                ys[:], c_sb["trow"][:], sy[:], selr[:, 0:1], op0=Alu.mult, op1=Alu.add
            )
            nc.vector.tensor_scalar(
                ys[:], ys[:], 0.0, 895.0, op0=Alu.max, op1=Alu.min
            )

            # ---------- x-side: xs rows [4,224] (clipped padded coords)
            sx = crp.tile([4, 1], F32, tag="sx")
            nc.vector.tensor_sub(sx[:], selr[:, 3:4], selr[:, 1:2])
            nc.vector.tensor_scalar(sx[:], sx[:], 1.0, None, op0=Alu.subtract)
            xs = crp.tile([4, OUT], F32, tag="xs")
            nc.vector.tensor_scalar(
                xs[:], c_sb["trow"][:], sx[:], selr[:, 1:2], op0=Alu.mult, op1=Alu.add
            )
            nc.vector.tensor_scalar(
                xs[:], xs[:], 0.0, 895.0, op0=Alu.max, op1=Alu.min
            )

            # ---------- per crop: vertical Wy matmul (V^T), horizontal Wx matmul
            for k in range(TOPN):
                touch(ys[0:1, 0:1])
                ysb_ps = psb.tile([112, OUT], F32, tag="big")
                nc.tensor.matmul(
                    ysb_ps[:],
                    lhsT=c_sb["pick4"][:, 112 * k : 112 * k + 112],
                    rhs=ys[:],
                    start=True,
                    stop=True,
                )
                wys = []
                for q in range(4):
                    dq = crp.tile([112, OUT], F32, tag=f"wyd{q}")
                    nc.scalar.activation(
                        dq[:], ysb_ps[:], Act.Abs,
                        bias=c_sb["uofs"][:, q : q + 1],
                    )
                    wq = crp.tile([112, OUT], F32, tag=f"wy{q}")
                    nc.scalar.activation(
                        wq[:], dq[:], Act.Relu, bias=1.0, scale=-1.0
                    )
                    wys.append(wq)
                touch(xs[0:1, 0:1])
                xsb_ps = psb.tile([112, OUT], F32, tag="big")
                nc.tensor.matmul(
                    xsb_ps[:],
                    lhsT=c_sb["pick4"][:, 112 * k : 112 * k + 112],
                    rhs=xs[:],
                    start=True,
                    stop=True,
                )
                wxs = []
                for q in range(4):
                    dq = crp.tile([112, OUT], F32, tag=f"wxd{q}")
                    nc.scalar.activation(
                        dq[:], xsb_ps[:], Act.Abs,
                        bias=c_sb["uofs"][:, q : q + 1],
                    )
                    wq = crp.tile([112, OUT], F32, tag=f"wx{q}")
                    nc.scalar.activation(
                        wq[:], dq[:], Act.Relu, bias=1.0, scale=-1.0
                    )
                    wxs.append(wq)
                for ch in range(3):
                    vts = []
                    for xc in range(4):
                        vt_ps = psb.tile([112, OUT], F32, tag="big")
                        for q in range(4):
                            nc.tensor.matmul(
                                vt_ps[:],
                                lhsT=imgY[q][
                                    :, 448 * ch + 112 * xc : 448 * ch + 112 * xc + 112
                                ],
                                rhs=wys[q][:],
                                start=(q == 0),
                                stop=(q == 3),
                            )
                        vt = rwp.tile([112, OUT], F32, tag=f"vt{xc}")
                        nc.vector.tensor_copy(vt[:], vt_ps[:])
                        touch(vt[0:1, 0:1])
                        vts.append(vt)
                    for c2 in range(2):
                        o_ps = psb.tile([112, OUT], F32, tag="big")
                        for xc in range(4):
                            nc.tensor.matmul(
                                o_ps[:],
                                lhsT=vts[xc][:, c2 * 112 : c2 * 112 + 112],
                                rhs=wxs[xc][:],
                                start=(xc == 0),
                                stop=(xc == 3),
                            )
                        o_ = hop.tile([112, OUT], F32, tag="o_")
                        nc.vector.tensor_copy(o_[:], o_ps[:])
                        nc.sync.dma_start(
                            part_out[k, ch, c2 * 112 : c2 * 112 + 112, :], o_[:]
                        )

            # ---------- small outputs
            idxi = np_.tile([1, TOPN], I32, tag="idxi")
            nc.vector.tensor_copy(idxi[:], idxrow[:])
            nc.sync.dma_start(idx_out[:], idxi[:])
            nc.sync.dma_start(prob_out[:], probrow[:])

    _legalize_waits(nc)
    return nc


def _legalize_waits(nc):
    """Walrus in this container only supports one sem-wait on most
    instruction encodings. Move excess waits onto same-engine Drain
    carriers (multi-wait Drains compile fine), and drop the tail SWDGE
    sem-reset InstISA (no SWDGE queues are used)."""
    import copy as _copy

    templates = {}
    for f in nc.m.functions:
        for bb in f.blocks:
            for ins in bb.instructions:
                if type(ins).__name__ == "InstDrain":
                    templates.setdefault(str(ins.engine), ins)
    for f in nc.m.functions:
        for bb in f.blocks:
            new = []
            for ins in bb.instructions:
                ty = type(ins).__name__
                if ty == "InstISA" and getattr(ins, "isa_opcode", None) == 176:
                    continue
                si = ins.sync_info
                if si is not None and si.on_wait and len(si.on_wait) > 1:
                    tmpl = templates.get(str(ins.engine))
                    if tmpl is not None:
                        waits = list(si.on_wait)
                        for wi, w in enumerate(waits[:-1]):
                            dr = _copy.replace(
                                tmpl, name=f"{ins.name}_wc{wi}"
                            )
                            dr.engine = ins.engine
                            dr.sync_info = mybir.SyncInfo(
                                on_wait=[w], on_update=[]
                            )
                            new.append(dr)
                        ins.sync_info = mybir.SyncInfo(
                            on_wait=[waits[-1]],
                            on_update=list(si.on_update or []),
                        )
                new.append(ins)
            bb.instructions = new
    return nc


_NC_CACHE = None


def kernel(x, rpn_feature, w1, b1, w2, b2, w3, b3, tw1, tb1, tw2, tb2, tw3, tb3,
           edge_anchors):
    global _NC_CACHE
    x = np.asarray(x, np.float32)
    rpn_feature = np.asarray(rpn_feature, np.float32)
    B = x.shape[0]
    assert B == NCORES

    if _NC_CACHE is None:
        _NC_CACHE = build_nc()
    nc = _NC_CACHE

    consts = _consts(np.asarray(edge_anchors))
    wts = _prep_weights(
        np.asarray(w1), np.asarray(b1), np.asarray(w2), np.asarray(b2),
        np.asarray(w3), np.asarray(b3), np.asarray(tw1), np.asarray(tb1),
        np.asarray(tw2), np.asarray(tb2), np.asarray(tw3), np.asarray(tb3),
    )
    shared = {}
    shared.update(consts)
    shared.update(wts)

    in_maps = []
    for b in range(B):
        im = dict(shared)
        rp_ = np.zeros((16, 128, 16, 16), np.float32)
        rp_[:, :, 1:15, 1:15] = rpn_feature[b].reshape(16, 128, 14, 14)
        im["rpn"] = rp_.reshape(16, 128, 256)
        im["xhcw"] = np.ascontiguousarray(
            x[b].transpose(1, 0, 2).reshape(448, 1344)
        )
        in_maps.append(im)

    res = run_bass_kernel_spmd(nc, in_maps, core_ids=list(range(NCORES)))
    parts = np.stack([res.results[b]["part"] for b in range(B)], axis=0)
    part_imgs = parts.reshape(B * TOPN, 3, OUT, OUT)
    top_n_prob = np.stack(
        [res.results[b]["prob"][0] for b in range(B)], axis=0
    ).astype(np.float32)
    top_n_index = np.stack(
        [res.results[b]["idx"][0] for b in range(B)], axis=0
    ).astype(np.int32)
    return part_imgs, top_n_prob, top_n_index
